# revision 39
# baseline (speedup 1.0000x reference)
"""Cost volume (tfa CorrelationCost, kernel_size=1, d=4) on 8 TRN2 cores.

out[b, k, y, x] = (1/C) * sum_c prv[b,c,y,x] * nxt_pad[b,c,y+dy,x+dx],
k = dy*9+dx, dy/dx in 0..8, nxt zero-padded by 4 on each spatial side.

Sharding: core i -> (batch b = i//2, H-half h = i%2). Each core gets the
full-C feature maps for its 64 rows (prv) and 72 padded rows (nxt).

Per-core algorithm (fp16 banded matmul), v10 final (53.9-59.3us
measured vs 93.3us v4 baseline). Pipeline structure driven by v4-v12
traces:

- All input DMAs ride the single gpsimd SWDGE queue in compute order;
  in-order draining gives the first tiles their data at full line rate
  instead of fair-sharing with later chunks (v4's first matmul waited
  25us; v8 starts ~12us).
- nxt is loaded in 4 column panels of 72 cols (8-col overlap, +9%
  bytes) x 3 row-chunks, so the first matmul needs only panel(0,0) +
  8 prv tiles = 0.5MB of input.
- Matmul pairs write [128, 2, 512] fp32 psum tiles (2 banks); one evac
  op (fp32->fp16 cast) moves both tiles, alternating vector/scalar per
  pair (31/33 split, ACT is ~6% faster): measured ~1142/1081 ns per
  pair -> ~287ns/tile steady pitch with both PSUM readers saturated.
  This is the compute wall: fp32 PSUM sources cap both readers at 1
  elem/cyc (16-bit psum would unlock 2x but is TRN3-only), per-op cost
  is AP-layout-invariant (v5/v6/v7), and concurrent SDMA reads of the
  stage cost ~+20%/op in SBUF bank contention (v8 vs v9) but beat
  serializing the output stream by far.
- stage[part, yb, wy, xb, wx] with pixel (q, r) on partition
  32*(q%4) + 4r + q//4. Output = 4 quad-DMAs per band: quad i reads
  partitions {i, i+4, ..., i+124} (stride 4 spans all 16 SBUF AXI
  ports; consecutive-32 blocks only reach 8 same-parity ports) and
  dumps wy rows [4i, 4i+12) as ONE contiguous 12.3KB run per partition
  (32 descriptors/DMA, line rate). Host picks the 9 needed rows.
- Out quads queue on the same SWDGE queue behind the inputs (bands 0-2
  drain during compute); band 3's four quads are spread across
  gpsimd/sync/scalar DGEs so their descriptor generation runs in
  parallel in the tail.

Traffic per core: prv 4.19MB + nxt 5.31MB + out 6.29MB = 15.8MB.
"""

import numpy as np

import bass_rust
import concourse.bass as bass
import concourse.tile as tile
from concourse import bacc, mybir
from concourse.bass_utils import run_bass_kernel_spmd

# Problem geometry (hardcoded per spec)
B, C, H, W = 4, 128, 128, 256
D = 4
ND = 2 * D + 1            # 9
K = ND * ND               # 81
HH = H // 2               # 64 rows per core
HP = HH + 2 * D           # 72 padded nxt rows per core
WP = W + 2 * D            # 264 padded nxt cols
YB, XB = 16, 8            # pixel tile: 16 rows x 8 cols = 128 partitions
NY, NX = YB + 2 * D, XB + 2 * D   # 24 x 16 window
NTY, NTX = HH // YB, W // XB      # 4 y-bands x 32 x-tiles
NWIN = NY * NX            # 384
N_CORES = 8
NP = 4                    # nxt column panels
PW = 72                   # panel width (64 + 8 halo)

ROW = NTY * NY * NTX * NX         # 49152 stage elems per partition
BAND = NY * NTX * NX              # 12288
QRUN = 12 * NTX * NX              # 6144 (quad slab: 12 wy rows x 32 xb x 16 wx)

F16 = mybir.dt.float16
F32 = mybir.dt.float32


def build_nc():
    nc = bacc.Bacc("TRN2")
    prv_d = nc.declare_dram_parameter("prv_s", [C, NTY * NTX * 128], F16, isOutput=False)
    nxt_d = nc.declare_dram_parameter("nxt_s", [C, NP * HP * PW], F16, isOutput=False)
    out_d = nc.declare_dram_parameter("out_s", [NTY, 4, 32, QRUN], F16, isOutput=True)

    with tile.TileContext(nc) as tc:
        with (
            tc.tile_pool(name="inp", bufs=1) as inp,
            tc.tile_pool(name="psum", bufs=4, space="PSUM") as pp,
            tc.tile_pool(name="stage", bufs=1) as sp,
        ):
            prv_sb = inp.tile([C, NTY * NTX * 128], F16)
            nxt_sb = inp.tile([C, NP, HP, PW], F16)
            # stage[part, yb, wy, xb, wx]: pixel (q, r) on partition
            # 32*(q%4) + 4r + q//4; its slab is wy rows [q, q+9).
            stage = sp.tile([128, NTY, NY, NTX, NX], F16)

            def nxt_chunk(j, p, eng=None):  # rows [24j, 24j+24) of panel p
                lo = (p * HP + 24 * j) * PW
                (eng or nc.gpsimd).dma_start(
                    nxt_sb[:, p, 24 * j : 24 * j + 24, :],
                    nxt_d[:, lo : lo + 24 * PW],
                )

            def prv_chunk(lo_t, n_t, eng=None):  # n_t tiles from tile lo_t
                lo = lo_t * 128
                (eng or nc.gpsimd).dma_start(
                    prv_sb[:, lo : lo + n_t * 128], prv_d[:, lo : lo + n_t * 128]
                )

            # Input order = compute order; single queue => in-order
            # completion at full bandwidth. (Splitting the first pair onto
            # the sync ring with a WAW handoff measured neutral-to-worse.)
            nxt_chunk(0, 0); prv_chunk(0, 4); prv_chunk(4, 4)
            nxt_chunk(0, 1); prv_chunk(8, 8)
            nxt_chunk(0, 2); prv_chunk(16, 8)
            nxt_chunk(0, 3); prv_chunk(24, 8)
            nxt_chunk(1, 0); nxt_chunk(1, 1); prv_chunk(32, 16)
            nxt_chunk(1, 2); nxt_chunk(1, 3); prv_chunk(48, 16)
            nxt_chunk(2, 0); nxt_chunk(2, 1); prv_chunk(64, 16)
            nxt_chunk(2, 2); nxt_chunk(2, 3); prv_chunk(80, 16)
            prv_chunk(96, 16); prv_chunk(112, 16)

            stage_t = stage[:, :, :, :, :].tensor

            for yb in range(NTY):
                # Absorb band-level input waits on cheap PE instructions.
                nc.tensor.ldweights(prv_sb[:, yb * NTX * 128 : yb * NTX * 128 + 1])
                nc.tensor.ldweights(nxt_sb[:, 0, 16 * yb, :1])
                nc.tensor.ldweights(nxt_sb[:, NP - 1, 16 * yb + 23, :1])
                for xp in range(NTX // 2):
                    ps = pp.tile([128, 2, 512], F32)
                    for t in range(2):
                        xb = 2 * xp + t
                        ti = yb * NTX + xb
                        lhsT = prv_sb[:, ti * 128 : (ti + 1) * 128]
                        p, co = xb >> 3, 8 * (xb & 7)
                        rhs = nxt_sb[:, p, yb * YB : yb * YB + NY, co : co + NX]
                        nc.tensor.matmul(ps[:, t, 0:NWIN], lhsT, rhs, start=True, stop=True)
                    # One evac per pair; strided psum src costs the same
                    # as any other AP shape (fixed ~+216ns/op, measured),
                    # so keep the stage dst slab-friendly. dst is a tile
                    # slice: raw-AP WRITES break Tile's range tracking
                    # (v6: out-DMAs serialized behind the last evac).
                    src = bass_rust.AP(
                        ps[:, :, :].tensor,
                        0,
                        [[2 * 512, 128], [NX, NY], [512, 2], [1, NX]],
                    )
                    dst = stage[:, yb, :, 2 * xp : 2 * xp + 2, :]
                    # 31/33 DVE/ACT split (ACT is ~6% faster per pair);
                    # the extra ACT pair sits mid-stream so the band-3
                    # finish stays balanced.
                    pi = yb * (NTX // 2) + xp
                    if pi % 2 == 0 and pi != 30:
                        nc.vector.tensor_copy(dst, src)
                    else:
                        nc.scalar.copy(dst, src)

            # Quad slab dump: quad i = partitions {i, i+4, ..., i+124}
            # (stride 4 spans all 16 SBUF AXI ports), one contiguous
            # 6144-elem slab per partition, expressed as a 3-dim AP of
            # 512-elem runs -- the ONLY form Tile's tracker handles
            # per-band (2-dim, or 3-dim with a count-2 middle dim, both
            # go conservative and serialize all outs behind the last
            # evac; v8/v13-measured). Bands 0-2 on the gpsimd queue, FIFO
            # behind the inputs; band 3 spread across engines.
            # Issue-chain balancing: each dma_start gens serially (~0.6us)
            # on its engine AFTER its sem wait, so a single engine carrying
            # all 16 quads issues band 3's only ~5us after the last evac
            # (v9-measured). Band 1 rides the idle sync ring (its ~3us of
            # input-stream contention is covered by band-3's input slack),
            # halving gpsimd's chain; band 3 issues three-way parallel.
            ENGS = {
                0: ["g", "g", "g", "g"],
                1: ["s", "s", "s", "s"],
                2: ["g", "g", "g", "g"],
                3: ["g", "s", "a", "g"],
            }
            for b in range(NTY):
                engs = [
                    {"g": nc.gpsimd, "s": nc.sync, "a": nc.scalar}[e]
                    for e in ENGS[b]
                ]
                for i in range(4):
                    if b < NTY - 1:
                        # 3-dim form: tracker-precise per-band deps, but
                        # 1KB descriptors (~220 GB/s drain).
                        dims = [[4 * ROW, 32], [NTX * NX, 12], [1, NTX * NX]]
                    else:
                        # Band 3 fires after the last evac anyway, so the
                        # conservative all-evac dep of the 2-dim form is
                        # free -- and its 12.3KB descriptors drain at line
                        # rate (~400 GB/s, v8-measured), -3.4us of tail.
                        dims = [[4 * ROW, 32], [1, QRUN]]
                    src = bass_rust.AP(
                        stage_t,
                        i * ROW + b * BAND + 4 * i * NTX * NX,
                        dims,
                    )
                    engs[i].dma_start(out_d[b, i], src)
    return nc


def make_in_maps(prv: np.ndarray, nxt: np.ndarray) -> list[dict[str, np.ndarray]]:
    prv = np.asarray(prv, dtype=np.float32)
    nxt = np.asarray(nxt, dtype=np.float32)
    nxt_pad = np.zeros((B, C, H + 2 * D, W + 2 * D), np.float32)
    nxt_pad[:, :, D : D + H, D : D + W] = nxt * np.float32(0.125)
    prv_s = prv * np.float32(0.0625)  # 2^-4 * 2^-3 = 1/C
    in_maps = []
    for core in range(N_CORES):
        b, h = divmod(core, 2)
        # prv tile-major, yb-outer; within a tile pixel (q, r) sits on
        # partition m = 32*(q%4) + 4*r + q//4 (port-spreading order for
        # the stride-4 quad out-DMAs): [C, yb, xb, q%4, r, q//4]
        p = prv_s[b, :, h * HH : (h + 1) * HH, :].reshape(C, NTY, 4, 4, NTX, XB)
        #                  axes: [C, yb, qh(4), ql(4), xb, r]
        p = np.ascontiguousarray(p.transpose(0, 1, 4, 3, 5, 2)).reshape(C, -1)
        # nxt in 4 column panels of 72 (8-col overlap): [C, panel, 72, 72]
        x = nxt_pad[b, :, h * HH : h * HH + HP, :]
        xp = np.stack([x[:, :, 64 * q : 64 * q + PW] for q in range(NP)], axis=1)
        in_maps.append(
            {
                "prv_s": p.astype(np.float16),
                "nxt_s": np.ascontiguousarray(xp).reshape(C, -1).astype(np.float16),
            }
        )
    return in_maps


def extract_core(O: np.ndarray) -> np.ndarray:
    """Quad slab dump -> [K, HH, W] fp32.

    O[band, quad, s, j*512 + xb*16 + wx] with s = 8*(q-4*quad)+r
    (partition 4*s + quad), j = wy - 4*quad, so dy = j - (q - 4*quad);
    displacement k=(dy,dx) is at wx = r + dx.
    """
    dy, dx = np.divmod(np.arange(K), ND)              # [81]
    r = np.arange(XB)
    A = np.asarray(O).astype(np.float32).reshape(NTY, 4, 4, XB, 12, NTX, NX)
    # A[band, quad, qq, r, j, xb, wx]
    G = A.transpose(0, 1, 2, 5, 3, 4, 6)              # [band, quad, qq, xb, r, j, wx]
    ridx = np.broadcast_to(r[None, :], (K, XB))
    wxidx = r[None, :] + dx[:, None]
    out = np.empty((NTY, 4, 4, NTX, K, XB), np.float32)
    for qq in range(4):
        jidx = np.broadcast_to(dy[:, None] + qq, (K, XB))
        out[:, :, qq] = G[:, :, qq][:, :, :, ridx, jidx, wxidx]
    T = out.transpose(4, 0, 1, 2, 3, 5)               # [81, band, quad, qq, xb, r]
    return T.reshape(K, HH, W)


def run(prv: np.ndarray, nxt: np.ndarray, trace: bool = False):
    nc = build_nc()
    nc.finalize()
    in_maps = make_in_maps(prv, nxt)
    res = run_bass_kernel_spmd(nc, in_maps, list(range(N_CORES)), trace=trace)
    out = np.empty((B, K, H, W), np.float32)
    for core in range(N_CORES):
        b, h = divmod(core, 2)
        out[b, :, h * HH : (h + 1) * HH, :] = extract_core(res.results[core]["out_s"])
    return out, res


def kernel(prv: np.ndarray, nxt: np.ndarray) -> np.ndarray:
    out, _ = run(prv, nxt, trace=False)
    return out


if __name__ == "__main__":
    rng = np.random.default_rng(0)
    prv = rng.standard_normal((B, C, H, W), dtype=np.float32)
    nxt = rng.standard_normal((B, C, H, W), dtype=np.float32)
    out = kernel(prv, nxt)
    print(out.shape, out.dtype)


# revision 40
# speedup vs baseline: 1.1112x; 1.1112x over previous
"""Cost volume (tfa CorrelationCost, kernel_size=1, d=4) on 8 TRN2 cores.

out[b, k, y, x] = (1/C) * sum_c prv[b,c,y,x] * nxt_pad[b,c,y+dy,x+dx],
k = dy*9+dx, dy/dx in 0..8, nxt zero-padded by 4 on each spatial side.

Sharding: core i -> (batch b = i//2, H-half h = i%2). Each core gets the
full-C feature maps for its 64 rows (prv) and 72 padded rows (nxt).

Per-core algorithm (fp16 banded matmul), v10 final (53.9-59.3us
measured vs 93.3us v4 baseline). Pipeline structure driven by v4-v12
traces:

- All input DMAs ride the single gpsimd SWDGE queue in compute order;
  in-order draining gives the first tiles their data at full line rate
  instead of fair-sharing with later chunks (v4's first matmul waited
  25us; v8 starts ~12us).
- nxt is loaded in 4 column panels of 72 cols (8-col overlap, +9%
  bytes) x 3 row-chunks, so the first matmul needs only panel(0,0) +
  8 prv tiles = 0.5MB of input.
- Matmul pairs write [128, 2, 512] fp32 psum tiles (2 banks); one evac
  op (fp32->fp16 cast) moves both tiles, alternating vector/scalar per
  pair (31/33 split, ACT is ~6% faster): measured ~1142/1081 ns per
  pair -> ~287ns/tile steady pitch with both PSUM readers saturated.
  This is the compute wall: fp32 PSUM sources cap both readers at 1
  elem/cyc (16-bit psum would unlock 2x but is TRN3-only), per-op cost
  is AP-layout-invariant (v5/v6/v7), and concurrent SDMA reads of the
  stage cost ~+20%/op in SBUF bank contention (v8 vs v9) but beat
  serializing the output stream by far.
- stage[part, yb, wy, xb, wx] with pixel (q, r) on partition
  32*(q%4) + 4r + q//4. Output = 4 quad-DMAs per band: quad i reads
  partitions {i, i+4, ..., i+124} (stride 4 spans all 16 SBUF AXI
  ports; consecutive-32 blocks only reach 8 same-parity ports) and
  dumps wy rows [4i, 4i+12) as ONE contiguous 12.3KB run per partition
  (32 descriptors/DMA, line rate). Host picks the 9 needed rows.
- Out quads queue on the same SWDGE queue behind the inputs (bands 0-2
  drain during compute); band 3's four quads are spread across
  gpsimd/sync/scalar DGEs so their descriptor generation runs in
  parallel in the tail.

Traffic per core: prv 4.19MB + nxt 5.31MB + out 6.29MB = 15.8MB.
"""

import numpy as np

import bass_rust
import concourse.bass as bass
import concourse.tile as tile
from concourse import bacc, mybir
from concourse.bass_utils import run_bass_kernel_spmd

# Problem geometry (hardcoded per spec)
B, C, H, W = 4, 128, 128, 256
D = 4
ND = 2 * D + 1            # 9
K = ND * ND               # 81
HH = H // 2               # 64 rows per core
HP = HH + 2 * D           # 72 padded nxt rows per core
WP = W + 2 * D            # 264 padded nxt cols
YB, XB = 16, 8            # pixel tile: 16 rows x 8 cols = 128 partitions
NY, NX = YB + 2 * D, XB + 2 * D   # 24 x 16 window
NTY, NTX = HH // YB, W // XB      # 4 y-bands x 32 x-tiles
NWIN = NY * NX            # 384
N_CORES = 8
NP = 4                    # nxt column panels
PW = 72                   # panel width (64 + 8 halo)

ROW = NTY * NY * NTX * NX         # 49152 stage elems per partition
BAND = NY * NTX * NX              # 12288
QRUN = 12 * NTX * NX              # 6144 (quad slab: 12 wy rows x 32 xb x 16 wx)

F16 = mybir.dt.float16
F32 = mybir.dt.float32


def build_nc():
    nc = bacc.Bacc("TRN2")
    prv_d = nc.declare_dram_parameter("prv_s", [C, NTY * NTX * 128], F16, isOutput=False)
    nxt_d = nc.declare_dram_parameter("nxt_s", [C, NP * HP * PW], F16, isOutput=False)
    out_d = nc.declare_dram_parameter("out_s", [NTY, 4, 32, QRUN], F16, isOutput=True)

    with tile.TileContext(nc) as tc:
        with (
            tc.tile_pool(name="inp", bufs=1) as inp,
            tc.tile_pool(name="psum", bufs=4, space="PSUM") as pp,
            tc.tile_pool(name="stage", bufs=1) as sp,
        ):
            prv_sb = inp.tile([C, NTY * NTX * 128], F16)
            nxt_sb = inp.tile([C, NP, HP, PW], F16)
            # stage[part, yb, wy, xb, wx]: pixel (q, r) on partition
            # 32*(q%4) + 4r + q//4; its slab is wy rows [q, q+9).
            stage = sp.tile([128, NTY, NY, NTX, NX], F16)

            def nxt_chunk(j, p, eng=None):  # rows [24j, 24j+24) of panel p
                lo = (p * HP + 24 * j) * PW
                (eng or nc.gpsimd).dma_start(
                    nxt_sb[:, p, 24 * j : 24 * j + 24, :],
                    nxt_d[:, lo : lo + 24 * PW],
                )

            def prv_chunk(lo_t, n_t, eng=None):  # n_t tiles from tile lo_t
                lo = lo_t * 128
                (eng or nc.gpsimd).dma_start(
                    prv_sb[:, lo : lo + n_t * 128], prv_d[:, lo : lo + n_t * 128]
                )

            # Input order = compute order; single queue => in-order
            # completion at full bandwidth. (Splitting the first pair onto
            # the sync ring with a WAW handoff measured neutral-to-worse.)
            nxt_chunk(0, 0); prv_chunk(0, 4); prv_chunk(4, 4)
            nxt_chunk(0, 1); prv_chunk(8, 8)
            nxt_chunk(0, 2); prv_chunk(16, 8)
            nxt_chunk(0, 3); prv_chunk(24, 8)
            nxt_chunk(1, 0); nxt_chunk(1, 1); prv_chunk(32, 16)
            nxt_chunk(1, 2); nxt_chunk(1, 3); prv_chunk(48, 16)
            nxt_chunk(2, 0); nxt_chunk(2, 1); prv_chunk(64, 16)
            nxt_chunk(2, 2); nxt_chunk(2, 3); prv_chunk(80, 16)
            prv_chunk(96, 16); prv_chunk(112, 16)

            stage_t = stage[:, :, :, :, :].tensor

            for yb in range(NTY):
                # Absorb band-level input waits on cheap PE instructions.
                nc.tensor.ldweights(prv_sb[:, yb * NTX * 128 : yb * NTX * 128 + 1])
                nc.tensor.ldweights(nxt_sb[:, 0, 16 * yb, :1])
                nc.tensor.ldweights(nxt_sb[:, NP - 1, 16 * yb + 23, :1])
                for xp in range(NTX // 2):
                    ps = pp.tile([128, 2, 512], F32)
                    for t in range(2):
                        xb = 2 * xp + t
                        ti = yb * NTX + xb
                        lhsT = prv_sb[:, ti * 128 : (ti + 1) * 128]
                        p, co = xb >> 3, 8 * (xb & 7)
                        rhs = nxt_sb[:, p, yb * YB : yb * YB + NY, co : co + NX]
                        nc.tensor.matmul(ps[:, t, 0:NWIN], lhsT, rhs, start=True, stop=True)
                    # One evac per pair; strided psum src costs the same
                    # as any other AP shape (fixed ~+216ns/op, measured),
                    # so keep the stage dst slab-friendly. dst is a tile
                    # slice: raw-AP WRITES break Tile's range tracking
                    # (v6: out-DMAs serialized behind the last evac).
                    src = bass_rust.AP(
                        ps[:, :, :].tensor,
                        0,
                        [[2 * 512, 128], [NX, NY], [512, 2], [1, NX]],
                    )
                    dst = stage[:, yb, :, 2 * xp : 2 * xp + 2, :]
                    # 31/33 DVE/ACT split (ACT is ~6% faster per pair);
                    # the extra ACT pair sits mid-stream so the band-3
                    # finish stays balanced.
                    pi = yb * (NTX // 2) + xp
                    if pi % 2 == 0 and pi != 30:
                        nc.vector.tensor_copy(dst, src)
                    else:
                        nc.scalar.copy(dst, src)

            # Quad slab dump: quad i = partitions {i, i+4, ..., i+124}
            # (stride 4 spans all 16 SBUF AXI ports), one contiguous
            # 6144-elem slab per partition, expressed as a 3-dim AP of
            # 512-elem runs -- the ONLY form Tile's tracker handles
            # per-band (2-dim, or 3-dim with a count-2 middle dim, both
            # go conservative and serialize all outs behind the last
            # evac; v8/v13-measured). Bands 0-2 on the gpsimd queue, FIFO
            # behind the inputs; band 3 spread across engines.
            # Issue-chain balancing: each dma_start gens serially (~0.6us)
            # on its engine AFTER its sem wait, so a single engine carrying
            # all 16 quads issues band 3's only ~5us after the last evac
            # (v9-measured). Band 1 rides the idle sync ring (its ~3us of
            # input-stream contention is covered by band-3's input slack),
            # halving gpsimd's chain; band 3 issues three-way parallel.
            ENGS = {
                0: ["g", "g", "g", "g"],
                1: ["s", "s", "s", "s"],
                2: ["g", "g", "g", "g"],
                3: ["g", "s", "a", "g"],
            }
            for b in range(NTY):
                engs = [
                    {"g": nc.gpsimd, "s": nc.sync, "a": nc.scalar}[e]
                    for e in ENGS[b]
                ]
                for i in range(4):
                    # 3-dim AP form throughout: tracker-precise per-band
                    # deps for bands 0-2. (Band 3 with 2-dim/12.3KB descs
                    # measured the same ~160-220 GB/s tail drain -- the
                    # limit is per-ring drain behavior, not descriptor
                    # size; v16-measured.)
                    src = bass_rust.AP(
                        stage_t,
                        i * ROW + b * BAND + 4 * i * NTX * NX,
                        [[4 * ROW, 32], [NTX * NX, 12], [1, NTX * NX]],
                    )
                    engs[i].dma_start(out_d[b, i], src)
    return nc


def make_in_maps(prv: np.ndarray, nxt: np.ndarray) -> list[dict[str, np.ndarray]]:
    prv = np.asarray(prv, dtype=np.float32)
    nxt = np.asarray(nxt, dtype=np.float32)
    nxt_pad = np.zeros((B, C, H + 2 * D, W + 2 * D), np.float32)
    nxt_pad[:, :, D : D + H, D : D + W] = nxt * np.float32(0.125)
    prv_s = prv * np.float32(0.0625)  # 2^-4 * 2^-3 = 1/C
    in_maps = []
    for core in range(N_CORES):
        b, h = divmod(core, 2)
        # prv tile-major, yb-outer; within a tile pixel (q, r) sits on
        # partition m = 32*(q%4) + 4*r + q//4 (port-spreading order for
        # the stride-4 quad out-DMAs): [C, yb, xb, q%4, r, q//4]
        p = prv_s[b, :, h * HH : (h + 1) * HH, :].reshape(C, NTY, 4, 4, NTX, XB)
        #                  axes: [C, yb, qh(4), ql(4), xb, r]
        p = np.ascontiguousarray(p.transpose(0, 1, 4, 3, 5, 2)).reshape(C, -1)
        # nxt in 4 column panels of 72 (8-col overlap): [C, panel, 72, 72]
        x = nxt_pad[b, :, h * HH : h * HH + HP, :]
        xp = np.stack([x[:, :, 64 * q : 64 * q + PW] for q in range(NP)], axis=1)
        in_maps.append(
            {
                "prv_s": p.astype(np.float16),
                "nxt_s": np.ascontiguousarray(xp).reshape(C, -1).astype(np.float16),
            }
        )
    return in_maps


def extract_core(O: np.ndarray) -> np.ndarray:
    """Quad slab dump -> [K, HH, W] fp32.

    O[band, quad, s, j*512 + xb*16 + wx] with s = 8*(q-4*quad)+r
    (partition 4*s + quad), j = wy - 4*quad, so dy = j - (q - 4*quad);
    displacement k=(dy,dx) is at wx = r + dx.
    """
    dy, dx = np.divmod(np.arange(K), ND)              # [81]
    r = np.arange(XB)
    A = np.asarray(O).astype(np.float32).reshape(NTY, 4, 4, XB, 12, NTX, NX)
    # A[band, quad, qq, r, j, xb, wx]
    G = A.transpose(0, 1, 2, 5, 3, 4, 6)              # [band, quad, qq, xb, r, j, wx]
    ridx = np.broadcast_to(r[None, :], (K, XB))
    wxidx = r[None, :] + dx[:, None]
    out = np.empty((NTY, 4, 4, NTX, K, XB), np.float32)
    for qq in range(4):
        jidx = np.broadcast_to(dy[:, None] + qq, (K, XB))
        out[:, :, qq] = G[:, :, qq][:, :, :, ridx, jidx, wxidx]
    T = out.transpose(4, 0, 1, 2, 3, 5)               # [81, band, quad, qq, xb, r]
    return T.reshape(K, HH, W)


def run(prv: np.ndarray, nxt: np.ndarray, trace: bool = False):
    nc = build_nc()
    nc.finalize()
    in_maps = make_in_maps(prv, nxt)
    res = run_bass_kernel_spmd(nc, in_maps, list(range(N_CORES)), trace=trace)
    out = np.empty((B, K, H, W), np.float32)
    for core in range(N_CORES):
        b, h = divmod(core, 2)
        out[b, :, h * HH : (h + 1) * HH, :] = extract_core(res.results[core]["out_s"])
    return out, res


def kernel(prv: np.ndarray, nxt: np.ndarray) -> np.ndarray:
    out, _ = run(prv, nxt, trace=False)
    return out


if __name__ == "__main__":
    rng = np.random.default_rng(0)
    prv = rng.standard_normal((B, C, H, W), dtype=np.float32)
    nxt = rng.standard_normal((B, C, H, W), dtype=np.float32)
    out = kernel(prv, nxt)
    print(out.shape, out.dtype)
